# revision 19
# baseline (speedup 1.0000x reference)
"""BlockSparseFFN (moe_routing) Trainium2 kernel — 8 NeuronCores, block-sparse.

The reference computes a dense SwiGLU then masks per (token, block): mask =
hard_top16 - stop_grad(sigmoid) + sigmoid, which is numerically exact 0/1
(IEEE: (0-p)+p == +0, (1-p)+p == 1 +/- 1ulp). So only 16 of 64 blocks are
live per token -> 4x FLOP reduction vs dense.

Strategy (tensor-parallel over blocks, per the sharding hint):
- Host: fp64 router -> per-token top-16 block set; per-block token index
  lists. Blocks (split into pieces for load balance) are sorted by size and
  packed into G groups x 8 cores with per-group capacities, so all cores run
  an identical (SPMD) schedule with ~3-5% slot padding.
- Host gathers x^T columns per piece (bf16) -> xgT per core; re-tiles the
  core's gate/up/down weight slices (bf16).
- Device (per core): for each group: load that group's weight set; stream
  token chunks (<=512): gate/up matmuls (bf16, f32 psum) over 16 k-tiles,
  silu*mul -> hidden bf16, down matmul per 128-token subtile into per-slot
  rows, write Dd[slot, 2048] bf16.
- Host: segment-sum the 16 per-pair rows per token (f32) -> output.
  (Pure data movement + 0.1% of FLOPs on host; all matmuls on device.)
"""
import sys

sys.path.insert(0, "/opt/trn_rl_repo")
import numpy as np
import ml_dtypes

import concourse.bass as bass
import concourse.mybir as mybir
import concourse.tile as tile
from concourse import bacc
from concourse.bass_utils import run_bass_kernel_spmd

N_CORES = 8
Bb, Ss, D = 4, 2048, 2048
N = Bb * Ss          # 8192 tokens
I = 8192             # intermediate
NB = 64              # blocks
BS = 128             # block size
TOP_K = 16
KT = D // 128        # 16 k-tiles (contraction for gate/up)
NDC = 4              # down d-chunks of 512

F32 = mybir.dt.float32
BF16 = mybir.dt.bfloat16
bf16 = ml_dtypes.bfloat16


def _chunks_of(cap):
    out = []
    r = cap
    while r > 0:
        c = min(512, r)
        out.append(c)
        r -= c
    return out


def build_nc(caps, repeat=1):
    caps = tuple(int(c) for c in caps)
    G = len(caps)
    S = sum(caps)
    WG = 3 * D  # per-group weight elements per partition (gw | uw | dw)
    nc = bacc.Bacc("TRN2", target_bir_lowering=False, debug=False,
                   num_devices=N_CORES)
    # xg: per-chunk SBUF-layout pack: [p, (chunk: k, t)] so one DMA per chunk
    # moves 128 contiguous KT*tn*2B descriptors.
    xg_d = nc.dram_tensor("xg", [128, KT * S], BF16, kind="ExternalInput")
    w_d = nc.dram_tensor("w", [128, G * WG], BF16, kind="ExternalInput")
    dd_d = nc.dram_tensor("dd", [S, D], BF16, kind="ExternalOutput")

    with tile.TileContext(nc) as tc:
        with tc.tile_pool(name="wp", bufs=3) as wp, \
             tc.tile_pool(name="xp", bufs=4) as xp, \
             tc.tile_pool(name="hp", bufs=3) as hp, \
             tc.tile_pool(name="sp", bufs=3) as sp, \
             tc.tile_pool(name="op", bufs=4) as op, \
             tc.tile_pool(name="pg", bufs=2, space="PSUM") as pg, \
             tc.tile_pool(name="pu", bufs=2, space="PSUM") as pu, \
             tc.tile_pool(name="pd", bufs=4, space="PSUM") as pd:
          for _rep in range(repeat):
            # flat software-pipelined chunk schedule: down(j-1) runs on PE
            # while scalar/vector produce hid(j-1) under gate/up(j)'s stream,
            # so the in-order PE never waits on the activation latency.
            chunks = []          # (slot_base, tn, group)
            base = 0
            for g in range(G):
                c0 = 0
                for tn in _chunks_of(caps[g]):
                    chunks.append((base + c0, tn, g))
                    c0 += tn
                base += caps[g]

            wts = {}

            def down_stage(prev):
                sb, tn, g = prev["sb"], prev["tn"], prev["g"]
                dwt = wts[g][:, 2 * D:3 * D]
                hid = prev["hid"]
                for ts in range((tn + 127) // 128):
                    w = min(128, tn - ts * 128)
                    ot = op.tile([128, D], BF16, tag="ot")
                    for dc in range(NDC):
                        pdt = pd.tile([128, 512], F32, tag="pd")
                        nc.tensor.matmul(pdt[:w, :],
                                         hid[:, ts * 128:ts * 128 + w],
                                         dwt[:, dc * 512:(dc + 1) * 512],
                                         start=True, stop=True)
                        if dc % 2 == 0:
                            nc.vector.tensor_copy(
                                ot[:w, dc * 512:(dc + 1) * 512], pdt[:w, :])
                        else:
                            nc.scalar.copy(
                                ot[:w, dc * 512:(dc + 1) * 512], pdt[:w, :])
                    nc.sync.dma_start(
                        dd_d.ap()[sb + ts * 128:sb + ts * 128 + w, :],
                        ot[:w, :])

            wts[0] = wp.tile([128, WG], BF16, tag="wt", name="wt0")
            nc.sync.dma_start(wts[0][:], w_d.ap()[:, 0:WG])
            prev = None
            for j, (sb, tn, g) in enumerate(chunks):
                xta = xp.tile([128, 8 * 512], BF16, tag="xta")
                nc.sync.dma_start(xta[:, :8 * tn],
                                  xg_d.ap()[:, KT * sb:KT * sb + 8 * tn])
                xtb = xp.tile([128, 8 * 512], BF16, tag="xtb")
                nc.sync.dma_start(xtb[:, :8 * tn],
                                  xg_d.ap()[:, KT * sb + 8 * tn:KT * (sb + tn)])
                if g + 1 < G and (g + 1) not in wts:
                    # prefetch next group's weights behind this chunk
                    wts[g + 1] = wp.tile([128, WG], BF16, tag="wt", name=f"wt{g + 1}")
                    nc.sync.dma_start(
                        wts[g + 1][:], w_d.ap()[:, (g + 1) * WG:(g + 2) * WG])
                gwt = wts[g][:, 0:D]
                uwt = wts[g][:, D:2 * D]
                pgt = pg.tile([128, 512], F32, tag="pg")
                for k in range(KT):
                    xs = xta if k < 8 else xtb
                    nc.tensor.matmul(pgt[:, :tn], gwt[:, k * 128:(k + 1) * 128],
                                     xs[:, (k % 8) * tn:(k % 8 + 1) * tn],
                                     start=(k == 0), stop=(k == KT - 1))
                put = pu.tile([128, 512], F32, tag="pu")
                for k in range(KT):
                    xs = xta if k < 8 else xtb
                    nc.tensor.matmul(put[:, :tn], uwt[:, k * 128:(k + 1) * 128],
                                     xs[:, (k % 8) * tn:(k % 8 + 1) * tn],
                                     start=(k == 0), stop=(k == KT - 1))
                sg = sp.tile([128, 512], F32, tag="sg")
                nc.scalar.activation(sg[:, :tn], pgt[:, :tn],
                                     mybir.ActivationFunctionType.Silu)
                hid = hp.tile([128, 512], BF16, tag="hid")
                nc.vector.tensor_mul(hid[:, :tn], sg[:, :tn], put[:, :tn])
                if prev is not None:
                    down_stage(prev)
                prev = {"sb": sb, "tn": tn, "g": g, "hid": hid}
            down_stage(prev)
    nc.compile()
    return nc


_CACHE = {}


def _get_nc(caps):
    key = tuple(caps)
    if key not in _CACHE:
        _CACHE[key] = build_nc(caps)
    return _CACHE[key]


def _host_mask_idx(x_flat, router_w1, router_w2):
    """fp64 router + top-16; returns per-block token index lists."""
    x64 = x_flat.astype(np.float64)
    r1 = x64 @ router_w1.astype(np.float64).T
    s = r1 / (1.0 + np.exp(-r1))
    lg = s @ router_w2.astype(np.float64).T          # [N, NB]
    kth = np.partition(lg, NB - TOP_K, axis=1)[:, NB - TOP_K:NB - TOP_K + 1]
    hard = lg >= kth                                  # [N, NB] bool
    return hard


def _plan_u(counts, u, roundto=32):
    """Cut blocks into pieces of ~u tokens, sort desc, group 8 at a time
    (one piece per core per group)."""
    pieces = []
    for b in range(NB):
        c = int(counts[b])
        n = max(1, -(-c // u))
        q, r = divmod(c, n)
        st = 0
        for j in range(n):
            ln = q + (1 if j < r else 0)
            pieces.append((b, st, ln))
            st += ln
    while len(pieces) % 8:
        pieces.append((0, 0, 0))
    pieces.sort(key=lambda t: -t[2])
    G = len(pieces) // 8
    caps, grid = [], []
    for g in range(G):
        grp = pieces[8 * g:8 * g + 8]
        cap = max(roundto,
                  int(np.ceil(max(t[2] for t in grp) / roundto)) * roundto)
        caps.append(cap)
        grid.append(grp)
    return caps, grid


def _plan(counts, G=None):
    """Pick the piece size u minimizing modeled DMA bytes."""
    counts = np.asarray(counts, np.int64)
    best = None
    for u in range(1024, 4400, 32):
        caps, grid = _plan_u(counts, u)
        cost = 2 * sum(caps) * D * 2 + len(caps) * 3 * D * 128 * 2
        if best is None or cost < best[0]:
            best = (cost, caps, grid)
    return best[1], best[2]


def prepare(x, gate_w, up_w, down_w, router_w1, router_w2, G=10):
    """Host prep: returns (in_maps, caps, perm, S) for the SPMD kernel."""
    x_flat = np.ascontiguousarray(np.asarray(x, np.float32)).reshape(N, D)
    hard = _host_mask_idx(x_flat, np.asarray(router_w1, np.float32),
                          np.asarray(router_w2, np.float32))
    counts = hard.sum(0)
    idx_by_block = [np.nonzero(hard[:, b])[0].astype(np.int64)
                    for b in range(NB)]
    caps, grid = _plan(counts, G)
    S = sum(caps)

    xT16 = np.ascontiguousarray(x_flat.astype(bf16).T)       # [D, N]
    g4 = np.asarray(gate_w, np.float32).reshape(NB, BS, KT, 128)  # b,i,k,p
    u4 = np.asarray(up_w, np.float32).reshape(NB, BS, KT, 128)
    d3 = np.asarray(down_w, np.float32).reshape(D, NB, BS)        # d,b,i

    in_maps = []
    tok_all = [[] for _ in range(N_CORES)]
    row_all = [[] for _ in range(N_CORES)]
    for c in range(N_CORES):
        idx_c = np.zeros(S, np.int64)
        w_c = np.empty((128, len(caps), 3 * D), bf16)  # p, g, (gw|uw|dw)
        base = 0
        for g, cap in enumerate(caps):
            b, st, ln = grid[g][c]
            ids = idx_by_block[b][st:st + ln]
            idx_c[base:base + ln] = ids
            tok_all[c].append(ids)
            row_all[c].append(c * S + base + np.arange(ln, dtype=np.int64))
            # gw tile [p, (k, i)] = gate_w[128*b + i, k*128 + p]
            w_c[:, g, 0:D] = g4[b].transpose(2, 1, 0).reshape(128, D).astype(bf16)
            w_c[:, g, D:2 * D] = u4[b].transpose(2, 1, 0).reshape(128, D).astype(bf16)
            w_c[:, g, 2 * D:3 * D] = d3[:, b, :].T.astype(bf16)   # i,d
            base += cap
        # pack gathered x into per-chunk SBUF layout [p, (chunk: k, t)]
        xg = xT16[:, idx_c]                                       # [D, S]
        parts = []
        base = 0
        for g, cap in enumerate(caps):
            c0 = 0
            for tn in _chunks_of(cap):
                blk = xg[:, base + c0:base + c0 + tn]             # [D, tn]
                parts.append(blk.reshape(KT, 128, tn)
                             .transpose(1, 0, 2).reshape(128, KT * tn))
                c0 += tn
            base += cap
        in_maps.append({
            "xg": np.ascontiguousarray(np.concatenate(parts, axis=1)),
            "w": np.ascontiguousarray(w_c.reshape(128, len(caps) * 3 * D)),
        })

    # permutation: for each token its 16 (core-relative) global Dd row ids
    toks = np.concatenate([t for c in range(N_CORES) for t in tok_all[c]])
    rows = np.concatenate([r for c in range(N_CORES) for r in row_all[c]])
    ordr = np.argsort(toks, kind="stable")
    perm = rows[ordr].reshape(N, TOP_K)
    return in_maps, caps, perm, S


def kernel(x, gate_w, up_w, down_w, router_w1, router_w2):
    in_maps, caps, perm, S = prepare(x, gate_w, up_w, down_w,
                                     router_w1, router_w2)
    nc = _get_nc(caps)
    res = run_bass_kernel_spmd(nc, in_maps, list(range(N_CORES)))
    dd_all = np.concatenate([res.results[c]["dd"] for c in range(N_CORES)],
                            axis=0)  # [8*S, 2048] bf16
    out = np.empty((N, D), np.float32)
    CH = 1024
    for t0 in range(0, N, CH):
        rows = perm[t0:t0 + CH].reshape(-1)
        out[t0:t0 + CH] = (dd_all[rows].astype(np.float32)
                           .reshape(-1, TOP_K, D).sum(1))
    return out.reshape(Bb, Ss, D)


# revision 20
# speedup vs baseline: 1.2967x; 1.2967x over previous
"""BlockSparseFFN (moe_routing) Trainium2 kernel — 8 NeuronCores, block-sparse.

The reference computes a dense SwiGLU then masks per (token, block): mask =
hard_top16 - stop_grad(sigmoid) + sigmoid, which is numerically exact 0/1
(IEEE: (0-p)+p == +0, (1-p)+p == 1 +/- 1ulp). So only 16 of 64 blocks are
live per token -> 4x FLOP reduction vs dense.

Strategy (tensor-parallel over blocks, per the sharding hint):
- Host: fp64 router -> per-token top-16 block set; per-block token index
  lists. Blocks (split into pieces for load balance) are sorted by size and
  packed into G groups x 8 cores with per-group capacities, so all cores run
  an identical (SPMD) schedule with ~3-5% slot padding.
- Host gathers x^T columns per piece (bf16) -> xgT per core; re-tiles the
  core's gate/up/down weight slices (bf16).
- Device (per core): for each group: load that group's weight set; stream
  token chunks (<=512): gate/up matmuls (bf16, f32 psum) over 16 k-tiles,
  silu*mul -> hidden bf16, down matmul per 128-token subtile into per-slot
  rows, write Dd[slot, 2048] bf16.
- Host: segment-sum the 16 per-pair rows per token (f32) -> output.
  (Pure data movement + 0.1% of FLOPs on host; all matmuls on device.)
"""
import sys

sys.path.insert(0, "/opt/trn_rl_repo")
import numpy as np
import ml_dtypes

import concourse.bass as bass
import concourse.mybir as mybir
import concourse.tile as tile
from concourse import bacc
from concourse.bass_utils import run_bass_kernel_spmd

N_CORES = 8
Bb, Ss, D = 4, 2048, 2048
N = Bb * Ss          # 8192 tokens
I = 8192             # intermediate
NB = 64              # blocks
BS = 128             # block size
TOP_K = 16
KT = D // 128        # 16 k-tiles (contraction for gate/up)
NDC = 4              # down d-chunks of 512

F32 = mybir.dt.float32
BF16 = mybir.dt.bfloat16
bf16 = ml_dtypes.bfloat16


def _chunks_of(cap):
    out = []
    r = cap
    while r > 0:
        c = min(512, r)
        out.append(c)
        r -= c
    return out


def build_nc(caps, repeat=1):
    caps = tuple(int(c) for c in caps)
    G = len(caps)
    S = sum(caps)
    WG = 3 * D  # per-group weight elements per partition (gw | uw | dw)
    nc = bacc.Bacc("TRN2", target_bir_lowering=False, debug=False,
                   num_devices=N_CORES)
    # xg: per-chunk SBUF-layout pack: [p, (chunk: k, t)] so one DMA per chunk
    # moves 128 contiguous KT*tn*2B descriptors.
    xg_d = nc.dram_tensor("xg", [128, KT * S], BF16, kind="ExternalInput")
    w_d = nc.dram_tensor("w", [128, G * WG], BF16, kind="ExternalInput")
    dd_d = nc.dram_tensor("dd", [S, D], BF16, kind="ExternalOutput")

    with tile.TileContext(nc) as tc:
        with tc.tile_pool(name="wp", bufs=3) as wp, \
             tc.tile_pool(name="xp", bufs=4) as xp, \
             tc.tile_pool(name="hp", bufs=3) as hp, \
             tc.tile_pool(name="sp", bufs=3) as sp, \
             tc.tile_pool(name="op", bufs=4) as op, \
             tc.tile_pool(name="pg", bufs=2, space="PSUM") as pg, \
             tc.tile_pool(name="pu", bufs=2, space="PSUM") as pu, \
             tc.tile_pool(name="pd", bufs=4, space="PSUM") as pd:
          if True:
            # flat software-pipelined chunk schedule across ALL repeats:
            # down(j-1) runs on PE while scalar/vector produce hid(j-1) under
            # gate/up(j)'s stream, so the in-order PE never waits on the
            # activation latency, and rep boundaries don't drain the pipeline.
            chunks = []          # (slot_base, tn, group_key)
            for rep in range(repeat):
                base = 0
                for g in range(G):
                    c0 = 0
                    for tn in _chunks_of(caps[g]):
                        chunks.append((base + c0, tn, (rep, g)))
                        c0 += tn
                    base += caps[g]
            gkeys = []
            for rep in range(repeat):
                gkeys.extend((rep, g) for g in range(G))
            nextkey = {k: gkeys[i + 1] for i, k in enumerate(gkeys[:-1])}

            wts = {}

            def down_stage(prev):
                sb, tn, g = prev["sb"], prev["tn"], prev["g"]
                dwt = wts[g][:, 2 * D:3 * D]  # g is a (rep, group) key
                hid = prev["hid"]
                for ts in range((tn + 127) // 128):
                    w = min(128, tn - ts * 128)
                    ot = op.tile([128, D], BF16, tag="ot")
                    for dc in range(NDC):
                        pdt = pd.tile([128, 512], F32, tag="pd")
                        nc.tensor.matmul(pdt[:w, :],
                                         hid[:, ts * 128:ts * 128 + w],
                                         dwt[:, dc * 512:(dc + 1) * 512],
                                         start=True, stop=True)
                        if dc % 2 == 0:
                            nc.vector.tensor_copy(
                                ot[:w, dc * 512:(dc + 1) * 512], pdt[:w, :])
                        else:
                            nc.scalar.copy(
                                ot[:w, dc * 512:(dc + 1) * 512], pdt[:w, :])
                    nc.sync.dma_start(
                        dd_d.ap()[sb + ts * 128:sb + ts * 128 + w, :],
                        ot[:w, :])

            wts[(0, 0)] = wp.tile([128, WG], BF16, tag="wt", name="wt0_0")
            nc.sync.dma_start(wts[(0, 0)][:], w_d.ap()[:, 0:WG])
            prev = None
            for j, (sb, tn, g) in enumerate(chunks):
                xta = xp.tile([128, 8 * 512], BF16, tag="xta")
                nc.sync.dma_start(xta[:, :8 * tn],
                                  xg_d.ap()[:, KT * sb:KT * sb + 8 * tn])
                xtb = xp.tile([128, 8 * 512], BF16, tag="xtb")
                nc.sync.dma_start(xtb[:, :8 * tn],
                                  xg_d.ap()[:, KT * sb + 8 * tn:KT * (sb + tn)])
                nk = nextkey.get(g)
                if nk is not None and nk not in wts:
                    # prefetch next group's weights behind this chunk
                    wts[nk] = wp.tile([128, WG], BF16, tag="wt",
                                      name=f"wt{nk[0]}_{nk[1]}")
                    nc.sync.dma_start(
                        wts[nk][:], w_d.ap()[:, nk[1] * WG:(nk[1] + 1) * WG])
                gwt = wts[g][:, 0:D]
                uwt = wts[g][:, D:2 * D]
                pgt = pg.tile([128, 512], F32, tag="pg")
                for k in range(KT):
                    xs = xta if k < 8 else xtb
                    nc.tensor.matmul(pgt[:, :tn], gwt[:, k * 128:(k + 1) * 128],
                                     xs[:, (k % 8) * tn:(k % 8 + 1) * tn],
                                     start=(k == 0), stop=(k == KT - 1))
                put = pu.tile([128, 512], F32, tag="pu")
                for k in range(KT):
                    xs = xta if k < 8 else xtb
                    nc.tensor.matmul(put[:, :tn], uwt[:, k * 128:(k + 1) * 128],
                                     xs[:, (k % 8) * tn:(k % 8 + 1) * tn],
                                     start=(k == 0), stop=(k == KT - 1))
                sg = sp.tile([128, 512], F32, tag="sg")
                nc.scalar.activation(sg[:, :tn], pgt[:, :tn],
                                     mybir.ActivationFunctionType.Silu)
                hid = hp.tile([128, 512], BF16, tag="hid")
                nc.vector.tensor_mul(hid[:, :tn], sg[:, :tn], put[:, :tn])
                if prev is not None:
                    down_stage(prev)
                prev = {"sb": sb, "tn": tn, "g": g, "hid": hid}
            down_stage(prev)
    nc.compile()
    return nc


_CACHE = {}


def _get_nc(caps):
    key = tuple(caps)
    if key not in _CACHE:
        _CACHE[key] = build_nc(caps)
    return _CACHE[key]


def _host_mask_idx(x_flat, router_w1, router_w2):
    """fp64 router + top-16; returns per-block token index lists."""
    x64 = x_flat.astype(np.float64)
    r1 = x64 @ router_w1.astype(np.float64).T
    s = r1 / (1.0 + np.exp(-r1))
    lg = s @ router_w2.astype(np.float64).T          # [N, NB]
    kth = np.partition(lg, NB - TOP_K, axis=1)[:, NB - TOP_K:NB - TOP_K + 1]
    hard = lg >= kth                                  # [N, NB] bool
    return hard


def _plan_u(counts, u, roundto=32):
    """Cut blocks into pieces of ~u tokens, sort desc, group 8 at a time
    (one piece per core per group)."""
    pieces = []
    for b in range(NB):
        c = int(counts[b])
        n = max(1, -(-c // u))
        q, r = divmod(c, n)
        st = 0
        for j in range(n):
            ln = q + (1 if j < r else 0)
            pieces.append((b, st, ln))
            st += ln
    while len(pieces) % 8:
        pieces.append((0, 0, 0))
    pieces.sort(key=lambda t: -t[2])
    G = len(pieces) // 8
    caps, grid = [], []
    for g in range(G):
        grp = pieces[8 * g:8 * g + 8]
        cap = max(roundto,
                  int(np.ceil(max(t[2] for t in grp) / roundto)) * roundto)
        caps.append(cap)
        grid.append(grp)
    return caps, grid


def _plan(counts, G=None):
    """Pick the piece size u minimizing modeled DMA bytes."""
    counts = np.asarray(counts, np.int64)
    best = None
    for u in range(1024, 4400, 32):
        caps, grid = _plan_u(counts, u)
        cost = 2 * sum(caps) * D * 2 + len(caps) * 3 * D * 128 * 2
        if best is None or cost < best[0]:
            best = (cost, caps, grid)
    return best[1], best[2]


def prepare(x, gate_w, up_w, down_w, router_w1, router_w2, G=10):
    """Host prep: returns (in_maps, caps, perm, S) for the SPMD kernel."""
    x_flat = np.ascontiguousarray(np.asarray(x, np.float32)).reshape(N, D)
    hard = _host_mask_idx(x_flat, np.asarray(router_w1, np.float32),
                          np.asarray(router_w2, np.float32))
    counts = hard.sum(0)
    idx_by_block = [np.nonzero(hard[:, b])[0].astype(np.int64)
                    for b in range(NB)]
    caps, grid = _plan(counts, G)
    S = sum(caps)

    xT16 = np.ascontiguousarray(x_flat.astype(bf16).T)       # [D, N]
    g4 = np.asarray(gate_w, np.float32).reshape(NB, BS, KT, 128)  # b,i,k,p
    u4 = np.asarray(up_w, np.float32).reshape(NB, BS, KT, 128)
    d3 = np.asarray(down_w, np.float32).reshape(D, NB, BS)        # d,b,i

    in_maps = []
    tok_all = [[] for _ in range(N_CORES)]
    row_all = [[] for _ in range(N_CORES)]
    for c in range(N_CORES):
        idx_c = np.zeros(S, np.int64)
        w_c = np.empty((128, len(caps), 3 * D), bf16)  # p, g, (gw|uw|dw)
        base = 0
        for g, cap in enumerate(caps):
            b, st, ln = grid[g][c]
            ids = idx_by_block[b][st:st + ln]
            idx_c[base:base + ln] = ids
            tok_all[c].append(ids)
            row_all[c].append(c * S + base + np.arange(ln, dtype=np.int64))
            # gw tile [p, (k, i)] = gate_w[128*b + i, k*128 + p]
            w_c[:, g, 0:D] = g4[b].transpose(2, 1, 0).reshape(128, D).astype(bf16)
            w_c[:, g, D:2 * D] = u4[b].transpose(2, 1, 0).reshape(128, D).astype(bf16)
            w_c[:, g, 2 * D:3 * D] = d3[:, b, :].T.astype(bf16)   # i,d
            base += cap
        # pack gathered x into per-chunk SBUF layout [p, (chunk: k, t)]
        xg = xT16[:, idx_c]                                       # [D, S]
        parts = []
        base = 0
        for g, cap in enumerate(caps):
            c0 = 0
            for tn in _chunks_of(cap):
                blk = xg[:, base + c0:base + c0 + tn]             # [D, tn]
                parts.append(blk.reshape(KT, 128, tn)
                             .transpose(1, 0, 2).reshape(128, KT * tn))
                c0 += tn
            base += cap
        in_maps.append({
            "xg": np.ascontiguousarray(np.concatenate(parts, axis=1)),
            "w": np.ascontiguousarray(w_c.reshape(128, len(caps) * 3 * D)),
        })

    # permutation: for each token its 16 (core-relative) global Dd row ids
    toks = np.concatenate([t for c in range(N_CORES) for t in tok_all[c]])
    rows = np.concatenate([r for c in range(N_CORES) for r in row_all[c]])
    ordr = np.argsort(toks, kind="stable")
    perm = rows[ordr].reshape(N, TOP_K)
    return in_maps, caps, perm, S


def kernel(x, gate_w, up_w, down_w, router_w1, router_w2):
    in_maps, caps, perm, S = prepare(x, gate_w, up_w, down_w,
                                     router_w1, router_w2)
    nc = _get_nc(caps)
    res = run_bass_kernel_spmd(nc, in_maps, list(range(N_CORES)))
    dd_all = np.concatenate([res.results[c]["dd"] for c in range(N_CORES)],
                            axis=0)  # [8*S, 2048] bf16
    out = np.empty((N, D), np.float32)
    CH = 1024
    for t0 in range(0, N, CH):
        rows = perm[t0:t0 + CH].reshape(-1)
        out[t0:t0 + CH] = (dd_all[rows].astype(np.float32)
                           .reshape(-1, TOP_K, D).sum(1))
    return out.reshape(Bb, Ss, D)


# revision 21
# speedup vs baseline: 1.3872x; 1.0698x over previous
"""BlockSparseFFN (moe_routing) Trainium2 kernel — 8 NeuronCores, block-sparse.

The reference computes a dense SwiGLU then masks per (token, block): mask =
hard_top16 - stop_grad(sigmoid) + sigmoid, which is numerically exact 0/1
(IEEE: (0-p)+p == +0, (1-p)+p == 1 +/- 1ulp). So only 16 of 64 blocks are
live per token -> 4x FLOP reduction vs dense.

Strategy (tensor-parallel over blocks, per the sharding hint):
- Host: fp64 router -> per-token top-16 block set; per-block token index
  lists. Blocks (split into pieces for load balance) are sorted by size and
  packed into G groups x 8 cores with per-group capacities, so all cores run
  an identical (SPMD) schedule with ~3-5% slot padding.
- Host gathers x^T columns per piece (bf16) -> xgT per core; re-tiles the
  core's gate/up/down weight slices (bf16).
- Device (per core): for each group: load that group's weight set; stream
  token chunks (<=512): gate/up matmuls (bf16, f32 psum) over 16 k-tiles,
  silu*mul -> hidden bf16, down matmul per 128-token subtile into per-slot
  rows, write Dd[slot, 2048] bf16.
- Host: segment-sum the 16 per-pair rows per token (f32) -> output.
  (Pure data movement + 0.1% of FLOPs on host; all matmuls on device.)
"""
import sys

sys.path.insert(0, "/opt/trn_rl_repo")
import numpy as np
import ml_dtypes

import concourse.bass as bass
import concourse.mybir as mybir
import concourse.tile as tile
from concourse import bacc
from concourse.bass_utils import run_bass_kernel_spmd

N_CORES = 8
Bb, Ss, D = 4, 2048, 2048
N = Bb * Ss          # 8192 tokens
I = 8192             # intermediate
NB = 64              # blocks
BS = 128             # block size
TOP_K = 16
KT = D // 128        # 16 k-tiles (contraction for gate/up)
NDC = 4              # down d-chunks of 512

F32 = mybir.dt.float32
BF16 = mybir.dt.bfloat16
bf16 = ml_dtypes.bfloat16


def _chunks_of(cap):
    out = []
    r = cap
    while r > 0:
        c = min(512, r)
        out.append(c)
        r -= c
    return out


def build_nc(caps, repeat=1):
    caps = tuple(int(c) for c in caps)
    G = len(caps)
    S = sum(caps)
    WG = 3 * D  # per-group weight elements per partition (gw | uw | dw)
    nc = bacc.Bacc("TRN2", target_bir_lowering=False, debug=False,
                   num_devices=N_CORES)
    # xg: per-chunk SBUF-layout pack: [p, (chunk: k, t)] so one DMA per chunk
    # moves 128 contiguous KT*tn*2B descriptors.
    xg_d = nc.dram_tensor("xg", [128, KT * S], BF16, kind="ExternalInput")
    w_d = nc.dram_tensor("w", [128, G * WG], BF16, kind="ExternalInput")
    dd_d = nc.dram_tensor("dd", [S, D], BF16, kind="ExternalOutput")

    with tile.TileContext(nc) as tc:
        with tc.tile_pool(name="wp", bufs=3) as wp, \
             tc.tile_pool(name="xp", bufs=4) as xp, \
             tc.tile_pool(name="hp", bufs=3) as hp, \
             tc.tile_pool(name="sp", bufs=3) as sp, \
             tc.tile_pool(name="op", bufs=4) as op, \
             tc.tile_pool(name="pg", bufs=2, space="PSUM") as pg, \
             tc.tile_pool(name="pu", bufs=2, space="PSUM") as pu, \
             tc.tile_pool(name="pd", bufs=4, space="PSUM") as pd:
          if True:
            # flat software-pipelined chunk schedule across ALL repeats:
            # down(j-1) runs on PE while scalar/vector produce hid(j-1) under
            # gate/up(j)'s stream, so the in-order PE never waits on the
            # activation latency, and rep boundaries don't drain the pipeline.
            chunks = []          # (slot_base, tn, group_key)
            for rep in range(repeat):
                base = 0
                for g in range(G):
                    c0 = 0
                    for tn in _chunks_of(caps[g]):
                        chunks.append((base + c0, tn, (rep, g)))
                        c0 += tn
                    base += caps[g]
            gkeys = []
            for rep in range(repeat):
                gkeys.extend((rep, g) for g in range(G))
            nextkey = {k: gkeys[i + 1] for i, k in enumerate(gkeys[:-1])}

            wts = {}

            def down_stage(prev):
                sb, tn, g = prev["sb"], prev["tn"], prev["g"]
                dwt = wts[g][:, 2 * D:3 * D]  # g is a (rep, group) key
                hid = prev["hid"]
                for ts in range((tn + 127) // 128):
                    w = min(128, tn - ts * 128)
                    ot = op.tile([128, D], BF16, tag="ot")
                    for dc in range(NDC):
                        pdt = pd.tile([128, 512], F32, tag="pd")
                        nc.tensor.matmul(pdt[:w, :],
                                         hid[:, ts * 128:ts * 128 + w],
                                         dwt[:, dc * 512:(dc + 1) * 512],
                                         start=True, stop=True)
                        if dc % 2 == 0:
                            nc.vector.tensor_copy(
                                ot[:w, dc * 512:(dc + 1) * 512], pdt[:w, :])
                        else:
                            nc.scalar.copy(
                                ot[:w, dc * 512:(dc + 1) * 512], pdt[:w, :])
                    nc.sync.dma_start(
                        dd_d.ap()[sb + ts * 128:sb + ts * 128 + w, :],
                        ot[:w, :])

            wts[(0, 0)] = wp.tile([128, WG], BF16, tag="wt", name="wt0_0")
            nc.sync.dma_start(wts[(0, 0)][:], w_d.ap()[:, 0:WG])
            prev = None
            for j, (sb, tn, g) in enumerate(chunks):
                xta = xp.tile([128, 8 * 512], BF16, tag="xta")
                nc.sync.dma_start(xta[:, :8 * tn],
                                  xg_d.ap()[:, KT * sb:KT * sb + 8 * tn])
                xtb = xp.tile([128, 8 * 512], BF16, tag="xtb")
                nc.scalar.dma_start(xtb[:, :8 * tn],
                                    xg_d.ap()[:, KT * sb + 8 * tn:KT * (sb + tn)])
                nk = nextkey.get(g)
                if nk is not None and nk not in wts:
                    # prefetch next group's weights behind this chunk
                    wts[nk] = wp.tile([128, WG], BF16, tag="wt",
                                      name=f"wt{nk[0]}_{nk[1]}")
                    nc.sync.dma_start(
                        wts[nk][:], w_d.ap()[:, nk[1] * WG:(nk[1] + 1) * WG])
                gwt = wts[g][:, 0:D]
                uwt = wts[g][:, D:2 * D]
                pgt = pg.tile([128, 512], F32, tag="pg")
                for k in range(KT):
                    xs = xta if k < 8 else xtb
                    nc.tensor.matmul(pgt[:, :tn], gwt[:, k * 128:(k + 1) * 128],
                                     xs[:, (k % 8) * tn:(k % 8 + 1) * tn],
                                     start=(k == 0), stop=(k == KT - 1))
                put = pu.tile([128, 512], F32, tag="pu")
                for k in range(KT):
                    xs = xta if k < 8 else xtb
                    nc.tensor.matmul(put[:, :tn], uwt[:, k * 128:(k + 1) * 128],
                                     xs[:, (k % 8) * tn:(k % 8 + 1) * tn],
                                     start=(k == 0), stop=(k == KT - 1))
                sg = sp.tile([128, 512], F32, tag="sg")
                nc.scalar.activation(sg[:, :tn], pgt[:, :tn],
                                     mybir.ActivationFunctionType.Silu)
                hid = hp.tile([128, 512], BF16, tag="hid")
                nc.vector.tensor_mul(hid[:, :tn], sg[:, :tn], put[:, :tn])
                if prev is not None:
                    down_stage(prev)
                prev = {"sb": sb, "tn": tn, "g": g, "hid": hid}
            down_stage(prev)
    nc.compile()
    return nc


_CACHE = {}


def _get_nc(caps):
    key = tuple(caps)
    if key not in _CACHE:
        _CACHE[key] = build_nc(caps)
    return _CACHE[key]


def _host_mask_idx(x_flat, router_w1, router_w2):
    """fp64 router + top-16; returns per-block token index lists."""
    x64 = x_flat.astype(np.float64)
    r1 = x64 @ router_w1.astype(np.float64).T
    s = r1 / (1.0 + np.exp(-r1))
    lg = s @ router_w2.astype(np.float64).T          # [N, NB]
    kth = np.partition(lg, NB - TOP_K, axis=1)[:, NB - TOP_K:NB - TOP_K + 1]
    hard = lg >= kth                                  # [N, NB] bool
    return hard


def _plan_u(counts, u, roundto=32):
    """Cut blocks into pieces of ~u tokens, sort desc, group 8 at a time
    (one piece per core per group)."""
    pieces = []
    for b in range(NB):
        c = int(counts[b])
        n = max(1, -(-c // u))
        q, r = divmod(c, n)
        st = 0
        for j in range(n):
            ln = q + (1 if j < r else 0)
            pieces.append((b, st, ln))
            st += ln
    while len(pieces) % 8:
        pieces.append((0, 0, 0))
    pieces.sort(key=lambda t: -t[2])
    G = len(pieces) // 8
    caps, grid = [], []
    for g in range(G):
        grp = pieces[8 * g:8 * g + 8]
        cap = max(roundto,
                  int(np.ceil(max(t[2] for t in grp) / roundto)) * roundto)
        caps.append(cap)
        grid.append(grp)
    return caps, grid


def _plan(counts, G=None):
    """Pick the piece size u minimizing modeled DMA bytes."""
    counts = np.asarray(counts, np.int64)
    best = None
    for u in range(1024, 4400, 32):
        caps, grid = _plan_u(counts, u)
        cost = 2 * sum(caps) * D * 2 + len(caps) * 3 * D * 128 * 2
        if best is None or cost < best[0]:
            best = (cost, caps, grid)
    return best[1], best[2]


def prepare(x, gate_w, up_w, down_w, router_w1, router_w2, G=10):
    """Host prep: returns (in_maps, caps, perm, S) for the SPMD kernel."""
    x_flat = np.ascontiguousarray(np.asarray(x, np.float32)).reshape(N, D)
    hard = _host_mask_idx(x_flat, np.asarray(router_w1, np.float32),
                          np.asarray(router_w2, np.float32))
    counts = hard.sum(0)
    idx_by_block = [np.nonzero(hard[:, b])[0].astype(np.int64)
                    for b in range(NB)]
    caps, grid = _plan(counts, G)
    S = sum(caps)

    xT16 = np.ascontiguousarray(x_flat.astype(bf16).T)       # [D, N]
    g4 = np.asarray(gate_w, np.float32).reshape(NB, BS, KT, 128)  # b,i,k,p
    u4 = np.asarray(up_w, np.float32).reshape(NB, BS, KT, 128)
    d3 = np.asarray(down_w, np.float32).reshape(D, NB, BS)        # d,b,i

    in_maps = []
    tok_all = [[] for _ in range(N_CORES)]
    row_all = [[] for _ in range(N_CORES)]
    for c in range(N_CORES):
        idx_c = np.zeros(S, np.int64)
        w_c = np.empty((128, len(caps), 3 * D), bf16)  # p, g, (gw|uw|dw)
        base = 0
        for g, cap in enumerate(caps):
            b, st, ln = grid[g][c]
            ids = idx_by_block[b][st:st + ln]
            idx_c[base:base + ln] = ids
            tok_all[c].append(ids)
            row_all[c].append(c * S + base + np.arange(ln, dtype=np.int64))
            # gw tile [p, (k, i)] = gate_w[128*b + i, k*128 + p]
            w_c[:, g, 0:D] = g4[b].transpose(2, 1, 0).reshape(128, D).astype(bf16)
            w_c[:, g, D:2 * D] = u4[b].transpose(2, 1, 0).reshape(128, D).astype(bf16)
            w_c[:, g, 2 * D:3 * D] = d3[:, b, :].T.astype(bf16)   # i,d
            base += cap
        # pack gathered x into per-chunk SBUF layout [p, (chunk: k, t)]
        xg = xT16[:, idx_c]                                       # [D, S]
        parts = []
        base = 0
        for g, cap in enumerate(caps):
            c0 = 0
            for tn in _chunks_of(cap):
                blk = xg[:, base + c0:base + c0 + tn]             # [D, tn]
                parts.append(blk.reshape(KT, 128, tn)
                             .transpose(1, 0, 2).reshape(128, KT * tn))
                c0 += tn
            base += cap
        in_maps.append({
            "xg": np.ascontiguousarray(np.concatenate(parts, axis=1)),
            "w": np.ascontiguousarray(w_c.reshape(128, len(caps) * 3 * D)),
        })

    # permutation: for each token its 16 (core-relative) global Dd row ids
    toks = np.concatenate([t for c in range(N_CORES) for t in tok_all[c]])
    rows = np.concatenate([r for c in range(N_CORES) for r in row_all[c]])
    ordr = np.argsort(toks, kind="stable")
    perm = rows[ordr].reshape(N, TOP_K)
    return in_maps, caps, perm, S


def kernel(x, gate_w, up_w, down_w, router_w1, router_w2):
    in_maps, caps, perm, S = prepare(x, gate_w, up_w, down_w,
                                     router_w1, router_w2)
    nc = _get_nc(caps)
    res = run_bass_kernel_spmd(nc, in_maps, list(range(N_CORES)))
    dd_all = np.concatenate([res.results[c]["dd"] for c in range(N_CORES)],
                            axis=0)  # [8*S, 2048] bf16
    out = np.empty((N, D), np.float32)
    CH = 1024
    for t0 in range(0, N, CH):
        rows = perm[t0:t0 + CH].reshape(-1)
        out[t0:t0 + CH] = (dd_all[rows].astype(np.float32)
                           .reshape(-1, TOP_K, D).sum(1))
    return out.reshape(Bb, Ss, D)
